# revision 22
# baseline (speedup 1.0000x reference)
"""Trainium2 Bass kernel for a GQA attention block (LuluAttention).

Problem: hidden_states [2, 2048, 2048], 16 q heads / 4 kv heads of dim 128,
RoPE, softmax attention, output projection.

Sharding: 8 cores = 2 (batch) x 4 (query-row blocks of 512 rows).
Each core computes Q for its 512-row slice (all 16 heads) and K/V for
ONLY that same 512-row seq quarter (so the whole phase reads just the
core's hsQ column slice -- no full-hsT load, no redundant K/V compute);
a DRAM AllGather over each 4-core batch group then assembles the full
roped K/V on every core, overlapped with the Q projection.  Attention
and the output projection follow per-core; the full output is a pure
host-side concatenation.

Key implementation choices (vs a straightforward fp32 version):
  - All DMA'd matmul operands (hs, Wq, Wk, Wv, Wo) are bf16: 1 PE
    cycle/row (4x over fp32) and half the HBM traffic.  On-device
    intermediates (q/k/v/attn/ctx) are also bf16; PSUM accumulation stays
    fp32.  Measured end-to-end rel err ~5e-3 (gate is 2e-2).
  - Everything is kept transposed ([head_dim, seq] with head_dim on SBUF
    partitions): QT/KT come straight out of matmul(lhsT=W_slice, rhs=hsT),
    scoresT = K @ Q^T, exp(scoresT) feeds the AV matmul directly
    (lhsT = V tile natural), denominator = ones @ expT, ctxT slices are
    directly the lhsT for the output projection.  No on-device transposes.
  - rotate_half for RoPE is a PE matmul with a constant +-1 permutation
    matrix R (rh = R^T @ x), replacing SBUF->SBUF half-swap DMAs; the
    K-quarter rows coincide with the Q rows so one table pair serves both.
  - The softmax denominator sums groups of 8 exp-tiles on the DVE first
    (bf16), leaving one ones-matmul per group on the PE.
  - The output projection is interleaved into the attention phase via a
    deferred-work queue (one matmul per attention step, accumulating in
    SBUF through a small PSUM ring), so there is no separate phase 3.
  - One PSUM pool set lives across all phases (pool re-allocation turns
    WAR deps into whole-phase engine barriers); bulk DMA rides a second
    HWDGE queue, chunked so critical transfers interleave; the collective
    and its pack/unpack run on the otherwise idle gpsimd queue.
  - Host packs all inputs into 5 tensors (each extra input buffer costs
    ~8-10us of per-exec dispatch overhead in this deployment).
"""

import sys

if "/opt/trn_rl_repo" not in sys.path:
    sys.path.insert(0, "/opt/trn_rl_repo")

import numpy as np

B, S, H = 2, 2048, 2048
NH, NKV, D = 16, 4, 128
SQ = 512          # query rows per core
NCORES = 8
P = 128
NT = H // P       # 16 contraction tiles over hidden dim
ST = S // P       # 16 seq tiles
ROPE_THETA = 10000.0
SCALE = 1.0 / float(np.sqrt(D))
GROUPS = NH // NKV


def _np_bf16():
    from concourse import mybir

    return mybir.dt.np(mybir.dt.bfloat16)


def _rope_tables_T():
    """cosT/sinT [D, S]: transposed plain RoPE tables (the rotate-half sign
    lives in the R permutation matrix, not the tables)."""
    inv_freq = 1.0 / (ROPE_THETA ** (np.arange(0, D, 2, dtype=np.float64) / D))
    t = np.arange(S, dtype=np.float64)
    freqs = np.outer(t, inv_freq)                     # [S, D/2]
    emb = np.concatenate([freqs, freqs], axis=-1)     # [S, D]
    cos = np.cos(emb).astype(np.float32)
    sin = np.sin(emb).astype(np.float32)
    return np.ascontiguousarray(cos.T), np.ascontiguousarray(sin.T)


def _rotate_half_matrix():
    """R [128, 128] with rh = R^T @ x == rotate_half(x) for x [d, n]:
    rh[m] = -x[m+64] for m<64, rh[m] = x[m-64] for m>=64."""
    R = np.zeros((D, D), dtype=np.float32)
    for m in range(D // 2):
        R[m + D // 2, m] = -1.0
    for m in range(D // 2, D):
        R[m - D // 2, m] = 1.0
    return R


def _build_program(mock_cc=False):
    from concourse import bacc, mybir, tile

    F32 = mybir.dt.float32
    BF16 = mybir.dt.bfloat16
    AF = mybir.ActivationFunctionType

    nc = bacc.Bacc(
        "TRN2", target_bir_lowering=False, debug=False, num_devices=NCORES
    )

    # big operands come in host-tiled [128, n*cols] layouts so each loads
    # with a single large DMA: x_t[p, i*cols + c] = x[i*128 + p, c]
    # inputs are consolidated into 5 tensors: each extra input buffer
    # costs ~8-10us of per-execution dispatch overhead in this runtime
    WQ_COLS = NT * NH * D
    WK_COLS = NT * NKV * D
    hsQ = nc.dram_tensor("hsQ", [P, NT * SQ], BF16, kind="ExternalInput").ap()
    wqo_t = nc.dram_tensor(
        "wqo", [P, WQ_COLS + NH * H], BF16, kind="ExternalInput"
    ).ap()
    wkv_t = nc.dram_tensor(
        "wkv", [P, 2 * WK_COLS], BF16, kind="ExternalInput"
    ).ap()
    # auxf cols: bqT[0:16] bkT[16:20] cosq[20:532] sinq[532:1044]
    auxf = nc.dram_tensor("auxf", [D, 1044], F32, kind="ExternalInput").ap()
    # auxb cols: rmat[0:128]; bv in row 0, cols [128:640]
    auxb = nc.dram_tensor("auxb", [P, 640], BF16, kind="ExternalInput").ap()
    wq = wqo_t[:, :WQ_COLS]
    wo = wqo_t[:, WQ_COLS:]
    wk = wkv_t[:, :WK_COLS]
    wv = wkv_t[:, WK_COLS:]
    bqT = auxf[:, 0:NH]
    bkT = auxf[:, NH : NH + NKV]
    cosq = auxf[:, 20:532]
    sinq = auxf[:, 532:1044]
    rmat = auxb[:, 0:D]
    bv = auxb[0:1, D : D + NKV * D]
    out = nc.dram_tensor("out", [SQ, H], F32, kind="ExternalOutput").ap()

    with tile.TileContext(nc) as tc:
        # ---- long-lived left-side pools ----
        cst = tc.alloc_tile_pool(name="cst", bufs=1)
        kvq = tc.alloc_tile_pool(name="kvq", bufs=1)
        ctxp = tc.alloc_tile_pool(name="ctxp", bufs=1)

        ones1 = cst.tile([1, P], BF16, tag="ones1")
        nc.gpsimd.memset(ones1[:], 1.0)
        ones128 = cst.tile([P, P], BF16, tag="ones128")
        nc.gpsimd.memset(ones128[:], 1.0)
        bqT_sb = cst.tile([D, NH], F32, tag="bqT")
        nc.scalar.dma_start(bqT_sb[:], bqT)
        bkT_sb = cst.tile([D, NKV], F32, tag="bkT")
        nc.scalar.dma_start(bkT_sb[:], bkT)
        bv_sb = cst.tile([1, NKV * D], BF16, tag="bv")
        nc.scalar.dma_start(bv_sb[:], bv)
        r_sb = cst.tile([D, D], BF16, tag="rmat")
        nc.scalar.dma_start(r_sb[:], rmat)
        cosq_sb = cst.tile([D, SQ], F32, tag="cosq")
        nc.scalar.dma_start(cosq_sb[:], cosq)
        sinq_sb = cst.tile([D, SQ], F32, tag="sinq")
        nc.scalar.dma_start(sinq_sb[:], sinq)

        # persistent bf16 intermediates
        qt = [kvq.tile([D, SQ], BF16, tag=f"qt{h}", name=f"qt{h}") for h in range(NH)]
        kt = [kvq.tile([D, S], BF16, tag=f"kt{g}", name=f"kt{g}") for g in range(NKV)]
        vt = [kvq.tile([P, NKV * D], BF16, tag=f"v{t}", name=f"v{t}") for t in range(ST)]
        ctx = [ctxp.tile([D, SQ], BF16, tag=f"ctx{h}", name=f"ctx{h}") for h in range(NH)]

        # ---- phase 1 ----
        # Each core computes K/V only for its own 512-row seq quarter
        # (which is exactly its hsQ slice), ropes it with the same tables
        # as Q, then an inter-core DRAM AllGather over the 4-core batch
        # group assembles the full K/V while the Q projection computes.
        # This removes the 4x-redundant K/V work AND the whole hsT load.
        hsp = tc.alloc_tile_pool(name="hsp", bufs=1, side="right")
        ws1 = tc.alloc_tile_pool(name="ws1", bufs=1, side="right")
        hsq = hsp.tile([P, NT * SQ], BF16, tag="hsq", name="hsq")

        # One PSUM pool set for ALL phases (3+3+2 banks): re-allocating
        # pools at a phase boundary coalesces the WAR dependency into an
        # engine-counter barrier over the whole previous phase.
        pa = tc.alloc_tile_pool(name="ps_a", bufs=3, space="PSUM")
        pc = tc.alloc_tile_pool(name="ps_c", bufs=3, space="PSUM")
        pob = tc.alloc_tile_pool(name="ps_ob", bufs=2, space="PSUM")
        rsc = tc.alloc_tile_pool(name="rsc", bufs=2, side="right")
        dcc = tc.alloc_tile_pool(name="dcc", bufs=1, space="DRAM")
        kv_in = dcc.tile([P, 8 * 512], BF16, tag="kvin", name="kv_in")
        kv_out = dcc.tile([4 * P, 8 * 512], BF16, tag="kvout", name="kv_out")

        def bank6(i, name):
            """Alternate pa/pc so up to 6 accumulators are live at once."""
            pool = pa if i % 2 == 0 else pc
            return pool.tile([P, 512], F32, tag="pj", name=name)

        def rope(dst, tbl_cos, tbl_sin):
            """dst [128, 512] bf16, in place; rh = R^T @ x on the PE."""
            rh = pob.tile([P, 512], F32, tag="po", name="rh")
            nc.tensor.matmul(rh[:], r_sb[:], dst[:], start=True, stop=True)
            t1 = rsc.tile([P, 512], F32, tag="rt1", bufs=2, name="rt1")
            nc.vector.tensor_mul(t1[:], rh[:], tbl_sin)
            t2 = rsc.tile([P, 512], F32, tag="rt2", bufs=2, name="rt2")
            nc.vector.tensor_mul(t2[:], dst[:], tbl_cos)
            nc.vector.tensor_add(dst[:], t1[:], t2[:])

        # -- K local quarter: kt_loc[g] [d, 512] = (hsq @ Wk + bk)^T --
        klo = [
            ws1.tile([D, 512], BF16, tag=f"klo{g}", name=f"klo{g}")
            for g in range(NKV)
        ]
        vlo = [
            ws1.tile([P, NKV * D], BF16, tag=f"vlo{t}", name=f"vlo{t}")
            for t in range(4)
        ]
        kbanks = {g: bank6(g, f"pk{g}") for g in range(NKV)}
        for ht in range(NT):
            nc.sync.dma_start(
                hsq[:, ht * SQ : (ht + 1) * SQ],
                hsQ[:, ht * SQ : (ht + 1) * SQ],
            )
            wkt = ws1.tile([P, NKV * D], BF16, tag="wk", bufs=3, name="wkt")
            nc.sync.dma_start(
                wkt[:], wk[:, ht * NKV * D : (ht + 1) * NKV * D]
            )
            for g in range(NKV):
                nc.tensor.matmul(
                    kbanks[g][:],
                    wkt[:, g * D : (g + 1) * D],
                    hsq[:, ht * SQ : (ht + 1) * SQ],
                    start=(ht == 0),
                    stop=(ht == NT - 1),
                )
        for g in range(NKV):
            nc.scalar.activation(
                klo[g][:], kbanks[g][:], AF.Identity,
                bias=bkT_sb[:, g : g + 1],
            )
            rope(klo[g][:], cosq_sb[:], sinq_sb[:])

        # -- V local quarter: vt_loc[ti] [128, 4*128] = hsq_sub @ Wv + bv --
        vbanks = {ti: bank6(ti, f"pv{ti}") for ti in range(4)}
        for ht in range(NT):
            wvt = ws1.tile([P, NKV * D], BF16, tag="wv", bufs=3, name="wvt")
            nc.sync.dma_start(
                wvt[:], wv[:, ht * NKV * D : (ht + 1) * NKV * D]
            )
            for ti in range(4):
                nc.tensor.matmul(
                    vbanks[ti][:],
                    hsq[:, ht * SQ + ti * P : ht * SQ + (ti + 1) * P],
                    wvt[:],
                    start=(ht == 0),
                    stop=False,
                )
        for ti in range(4):
            nc.tensor.matmul(
                vbanks[ti][:], ones1[:], bv_sb[:], start=False, stop=True
            )
            nc.scalar.copy(vlo[ti][:], vbanks[ti][:])

        # -- gather: pack -> AllGather over the batch group -> unpack.
        # All on the (otherwise idle) Pool/gpsimd queue; the Q projection
        # below overlaps the whole exchange.
        for g in range(NKV):
            nc.gpsimd.dma_start(kv_in[:, g * 512 : (g + 1) * 512], klo[g][:])
        for ti in range(4):
            nc.gpsimd.dma_start(
                kv_in[:, (4 + ti) * 512 : (5 + ti) * 512], vlo[ti][:]
            )
        if mock_cc:
            # sim-only stand-in with the same DMA volume (TimelineSim
            # cannot run collectives): broadcast own quarter to all blocks
            for k in range(4):
                nc.gpsimd.dma_start(kv_out[k * P : (k + 1) * P, :], kv_in[:])
        else:
            nc.gpsimd.collective_compute(
                "AllGather",
                mybir.AluOpType.bypass,
                replica_groups=[[0, 1, 2, 3], [4, 5, 6, 7]],
                ins=[kv_in[:].opt()],
                outs=[kv_out[:].opt()],
            )
        for g in range(NKV):
            for k in range(4):
                nc.gpsimd.dma_start(
                    kt[g][:, k * 512 : (k + 1) * 512],
                    kv_out[k * P : (k + 1) * P, g * 512 : (g + 1) * 512],
                )
        for ti in range(4):
            for k in range(4):
                nc.gpsimd.dma_start(
                    vt[k * 4 + ti][:],
                    kv_out[k * P : (k + 1) * P, (4 + ti) * 512 : (5 + ti) * 512],
                )

        # -- Q: three sweeps of <=6 heads; wq streamed in per-sweep chunks --
        for h0, h1 in ((0, 6), (6, 12), (12, 16)):
            banks = {
                h: bank6(h - h0, f"pq{h}") for h in range(h0, h1)
            }
            for ht in range(NT):
                wqt = ws1.tile(
                    [P, 6 * P], BF16, tag="wq", bufs=3, name="wqt"
                )
                nc.sync.dma_start(
                    wqt[:, : (h1 - h0) * P],
                    wq[:, ht * NH * D + h0 * D : ht * NH * D + h1 * D],
                )
                for h in range(h0, h1):
                    nc.tensor.matmul(
                        banks[h][:],
                        wqt[:, (h - h0) * P : (h - h0 + 1) * P],
                        hsq[:, ht * SQ : (ht + 1) * SQ],
                        start=(ht == 0),
                        stop=(ht == NT - 1),
                    )
            for h in range(h0, h1):
                nc.scalar.activation(
                    qt[h][:], banks[h][:], AF.Identity,
                    bias=bqT_sb[:, h : h + 1],
                )
                rope(qt[h][:], cosq_sb[:], sinq_sb[:])
        rsc.release()
        ws1.release()
        hsp.release()

        # ---- phase 2: attention; wo prefetch streams during it ----
        wop = tc.alloc_tile_pool(name="wop", bufs=1, side="right")
        wores = wop.tile([P, NH * H], BF16, tag="wo", name="wores")
        for i in range(4):
            cs = NH * H // 4
            nc.sync.dma_start(
                wores[:, i * cs : (i + 1) * cs], wo[:, i * cs : (i + 1) * cs]
            )

        # out_acc[s1t] [128, 2048] f32 accumulates the output projection in
        # SBUF; groups of 4 heads go PSUM -> (DVE add) -> out_acc, so the
        # out-proj matmuls ride the PE's slack in the Act-bound attention
        # phase and there is no separate phase 3.
        oacc = tc.alloc_tile_pool(name="oacc", bufs=1, side="right")
        out_acc = [
            oacc.tile([P, H], F32, tag=f"oa{s}", name=f"oa{s}")
            for s in range(4)
        ]

        wsa = tc.alloc_tile_pool(name="wsa", bufs=4, side="right")
        DG = 8  # den group: sum DG exp-tiles on the DVE, 1 den matmul/group

        # Deferred out-proj emitters: each is one PE matmul (or one DVE
        # drain); they are popped one per (h, t) step so they fill the PE's
        # slack without ever blocking the Act-engine exp stream.
        pending = []

        def push_group_out(h):
            """Queue out-proj work for finished heads h-3..h."""
            for s1t in range(4):
                for hc in range(4):
                    bank_box = []

                    def mk(hh, hc=hc, s1t=s1t, h=h, bank_box=bank_box):
                        def emit():
                            if not bank_box:
                                bank_box.append(
                                    pob.tile(
                                        [P, 512], F32, tag="po", name="po"
                                    )
                                )
                            nc.tensor.matmul(
                                bank_box[0][:],
                                ctx[hh][:, s1t * P : (s1t + 1) * P],
                                wores[
                                    :,
                                    hh * H + hc * 512 : hh * H + (hc + 1) * 512,
                                ],
                                start=(hh == h - 3),
                                stop=(hh == h),
                            )

                        return emit

                    for hh in range(h - 3, h + 1):
                        emit = mk(hh)
                        emit.is_mm = True
                        pending.append(emit)

                    def drain(hc=hc, s1t=s1t, h=h, bank_box=bank_box):
                        dst = out_acc[s1t][:, hc * 512 : (hc + 1) * 512]
                        if h == 3:
                            nc.vector.tensor_copy(dst, bank_box[0][:])
                        else:
                            nc.vector.tensor_add(dst, dst, bank_box[0][:])

                    pending.append(drain)
                    if h == NH - 1:
                        def outdma(s1t=s1t, hc=hc):
                            nc.sync.dma_start(
                                out[
                                    s1t * P : (s1t + 1) * P,
                                    hc * 512 : (hc + 1) * 512,
                                ],
                                out_acc[s1t][:, hc * 512 : (hc + 1) * 512],
                            )

                        pending.append(outdma)

        def pop_pending():
            """Emit queued DVE drains freely plus one PE matmul."""
            while pending:
                fn = pending.pop(0)
                fn()
                if getattr(fn, "is_mm", False):
                    break

        for h in range(NH):
            g = h // GROUPS
            ctx_ps = pc.tile([P, SQ], F32, tag="pj", name=f"ctxps{h}")
            den_ps = pc.tile([P, SQ], F32, tag="pj", name=f"denps{h}")
            sc = [None] * ST

            def score(t):
                sc[t] = pa.tile([P, SQ], F32, tag="pj", name="sc")
                nc.tensor.matmul(
                    sc[t][:],
                    kt[g][:, t * P : (t + 1) * P],
                    qt[h][:],
                    start=True,
                    stop=True,
                )

            score(0)
            asum = None
            for t in range(ST):
                at = wsa.tile([P, SQ], BF16, tag="at", name="at")
                nc.scalar.activation(at[:], sc[t][:], AF.Exp, scale=SCALE)
                if t + 1 < ST:
                    score(t + 1)
                nc.tensor.matmul(
                    ctx_ps[:],
                    vt[t][:, g * D : (g + 1) * D],
                    at[:],
                    start=(t == 0),
                    stop=(t == ST - 1),
                )
                if t % DG == 0:
                    at0 = at
                elif t % DG == 1:
                    asum = wsa.tile([P, SQ], BF16, tag="as", bufs=2, name="asum")
                    nc.vector.tensor_add(asum[:], at0[:], at[:])
                else:
                    nc.vector.tensor_add(asum[:], asum[:], at[:])
                if t % DG == DG - 1:
                    nc.tensor.matmul(
                        den_ps[:],
                        ones128[:],
                        asum[:],
                        start=(t == DG - 1),
                        stop=(t == ST - 1),
                    )
                pop_pending()
            rc = wsa.tile([P, SQ], F32, tag="rc", bufs=2, name="rc")
            nc.vector.reciprocal(rc[:], den_ps[:])
            nc.vector.tensor_mul(ctx[h][:], ctx_ps[:], rc[:])

            if h % 4 == 3:
                push_group_out(h)
        while pending:
            pending.pop(0)()
        wsa.release()
        oacc.release()
        wop.release()
        dcc.release()
        pob.release()
        pc.release()
        pa.release()
        ctxp.release()
        kvq.release()
        cst.release()

    nc.compile()
    return nc


_PROGRAM_CACHE = {}


def _get_program():
    if "nc" not in _PROGRAM_CACHE:
        _PROGRAM_CACHE["nc"] = _build_program()
    return _PROGRAM_CACHE["nc"]


def _tile_rows(x_b):
    """[n*128, cols] -> [128, n*cols] with x_t[p, i*cols + c] = x[i*128+p, c]."""
    n = x_b.shape[0] // P
    cols = x_b.shape[1]
    return np.ascontiguousarray(
        x_b.reshape(n, P, cols).transpose(1, 0, 2).reshape(P, n * cols)
    )


def _prepare_in_maps(hidden_states, Wq, bq, Wk, bk, Wv, bv, Wo):
    bf16 = _np_bf16()
    hidden_states = np.asarray(hidden_states, dtype=np.float32)
    Wq_t = _tile_rows(np.asarray(Wq, dtype=np.float32).astype(bf16))
    Wk_t = _tile_rows(np.asarray(Wk, dtype=np.float32).astype(bf16))
    Wv_t = _tile_rows(np.asarray(Wv, dtype=np.float32).astype(bf16))
    Wo_t = _tile_rows(np.asarray(Wo, dtype=np.float32).astype(bf16))
    wqo_h = np.concatenate([Wq_t, Wo_t], axis=1)
    wkv_h = np.concatenate([Wk_t, Wv_t], axis=1)
    bq = np.asarray(bq, dtype=np.float32)
    bk = np.asarray(bk, dtype=np.float32)
    bv_b = np.asarray(bv, dtype=np.float32).astype(bf16).reshape(1, NKV * D)

    cosT, sinT = _rope_tables_T()
    rmat = _rotate_half_matrix().astype(bf16)
    bqT_h = np.ascontiguousarray(bq.reshape(NH, D).T)    # [128, 16]
    bkT_h = np.ascontiguousarray(bk.reshape(NKV, D).T)   # [128, 4]
    auxb_h = np.zeros((P, 640), bf16)
    auxb_h[:, :D] = rmat
    auxb_h[0:1, D : D + NKV * D] = bv_b

    hsT_b = [
        np.ascontiguousarray(hidden_states[b].T).astype(bf16) for b in range(B)
    ]

    in_maps = []
    for core in range(NCORES):
        b, tq = core // 4, core % 4
        qoff = tq * SQ
        auxf_h = np.zeros((D, 1044), np.float32)
        auxf_h[:, :NH] = bqT_h
        auxf_h[:, NH : NH + NKV] = bkT_h
        auxf_h[:, 20:532] = cosT[:, qoff : qoff + SQ]
        auxf_h[:, 532:1044] = sinT[:, qoff : qoff + SQ]
        in_maps.append(
            {
                "hsQ": _tile_rows(
                    np.ascontiguousarray(hsT_b[b][:, qoff : qoff + SQ])
                ),
                "wqo": wqo_h,
                "wkv": wkv_h,
                "auxf": auxf_h,
                "auxb": auxb_h,
            }
        )
    return in_maps


def kernel(hidden_states, Wq, bq, Wk, bk, Wv, bv, Wo):
    from concourse.bass_utils import run_bass_kernel_spmd

    in_maps = _prepare_in_maps(hidden_states, Wq, bq, Wk, bk, Wv, bv, Wo)
    nc = _get_program()
    res = run_bass_kernel_spmd(
        nc, in_maps, core_ids=list(range(NCORES)), trace=False
    )

    out_full = np.empty((B, S, H), dtype=np.float32)
    for core in range(NCORES):
        b, tq = core // 4, core % 4
        out_full[b, tq * SQ : (tq + 1) * SQ, :] = res.results[core]["out"]
    return out_full


# revision 30
# speedup vs baseline: 1.2124x; 1.2124x over previous
"""Trainium2 Bass kernel for a GQA attention block (LuluAttention).

Problem: hidden_states [2, 2048, 2048], 16 q heads / 4 kv heads of dim 128,
RoPE, softmax attention, output projection.

Sharding: 8 cores = 2 (batch) x 4 (query-row blocks of 512 rows).
Each core computes Q for its 512-row slice (all 16 heads) and K/V for
ONLY that same 512-row seq quarter (so the whole phase reads just the
core's hsQ column slice -- no full-hsT load, no redundant K/V compute);
a DRAM AllGather over each 4-core batch group then assembles the full
roped K/V on every core, overlapped with the Q projection.  Attention
and the output projection follow per-core; the full output is a pure
host-side concatenation.

Key implementation choices (vs a straightforward fp32 version):
  - All DMA'd matmul operands (hs, Wq, Wk, Wv, Wo) are bf16: 1 PE
    cycle/row (4x over fp32) and half the HBM traffic.  On-device
    intermediates (q/k/v/attn/ctx) are also bf16; PSUM accumulation stays
    fp32.  Measured end-to-end rel err ~5e-3 (gate is 2e-2).
  - Everything is kept transposed ([head_dim, seq] with head_dim on SBUF
    partitions): QT/KT come straight out of matmul(lhsT=W_slice, rhs=hsT),
    scoresT = K @ Q^T, exp(scoresT) feeds the AV matmul directly
    (lhsT = V tile natural), denominator = ones @ expT, ctxT slices are
    directly the lhsT for the output projection.  No on-device transposes.
  - rotate_half for RoPE is a PE matmul with a constant +-1 permutation
    matrix R (rh = R^T @ x), replacing SBUF->SBUF half-swap DMAs; the
    K-quarter rows coincide with the Q rows so one table pair serves both.
  - The softmax denominator sums groups of 8 exp-tiles on the DVE first
    (bf16), leaving one ones-matmul per group on the PE.
  - The output projection is interleaved into the attention phase via a
    deferred-work queue (one matmul per attention step, accumulating in
    SBUF through a small PSUM ring), so there is no separate phase 3.
  - One PSUM pool set lives across all phases (pool re-allocation turns
    WAR deps into whole-phase engine barriers); bulk DMA rides a second
    HWDGE queue, chunked so critical transfers interleave; the collective
    and its pack/unpack run on the otherwise idle gpsimd queue.
  - Host packs all inputs into 5 tensors (each extra input buffer costs
    ~8-10us of per-exec dispatch overhead in this deployment).
"""

import sys

if "/opt/trn_rl_repo" not in sys.path:
    sys.path.insert(0, "/opt/trn_rl_repo")

import numpy as np

B, S, H = 2, 2048, 2048
NH, NKV, D = 16, 4, 128
SQ = 512          # query rows per core
NCORES = 8
P = 128
NT = H // P       # 16 contraction tiles over hidden dim
ST = S // P       # 16 seq tiles
ROPE_THETA = 10000.0
SCALE = 1.0 / float(np.sqrt(D))
GROUPS = NH // NKV


def _np_bf16():
    from concourse import mybir

    return mybir.dt.np(mybir.dt.bfloat16)


def _rope_tables_T():
    """cosT/sinT [D, S]: transposed plain RoPE tables (the rotate-half sign
    lives in the R permutation matrix, not the tables)."""
    inv_freq = 1.0 / (ROPE_THETA ** (np.arange(0, D, 2, dtype=np.float64) / D))
    t = np.arange(S, dtype=np.float64)
    freqs = np.outer(t, inv_freq)                     # [S, D/2]
    emb = np.concatenate([freqs, freqs], axis=-1)     # [S, D]
    cos = np.cos(emb).astype(np.float32)
    sin = np.sin(emb).astype(np.float32)
    return np.ascontiguousarray(cos.T), np.ascontiguousarray(sin.T)


def _rotate_half_matrix():
    """R [128, 128] with rh = R^T @ x == rotate_half(x) for x [d, n]:
    rh[m] = -x[m+64] for m<64, rh[m] = x[m-64] for m>=64."""
    R = np.zeros((D, D), dtype=np.float32)
    for m in range(D // 2):
        R[m + D // 2, m] = -1.0
    for m in range(D // 2, D):
        R[m - D // 2, m] = 1.0
    return R


def _build_program(mock_cc=False):
    from concourse import bacc, mybir, tile

    F32 = mybir.dt.float32
    BF16 = mybir.dt.bfloat16
    AF = mybir.ActivationFunctionType

    nc = bacc.Bacc(
        "TRN2", target_bir_lowering=False, debug=False, num_devices=NCORES
    )

    # big operands come in host-tiled [128, n*cols] layouts so each loads
    # with a single large DMA: x_t[p, i*cols + c] = x[i*128 + p, c]
    # all bf16 inputs live in ONE tensor, f32 in another: each extra
    # input buffer costs ~10us of per-execution dispatch overhead in this
    # runtime, so 2 buffers beat 5
    WQ_COLS = NT * NH * D
    WK_COLS = NT * NKV * D
    HSQ_COLS = NT * SQ
    # inb columns: hsQ | wk | wv | wq | wo | auxb(rmat, bv)
    OFF_WK = HSQ_COLS
    OFF_WV = OFF_WK + WK_COLS
    OFF_WQ = OFF_WV + WK_COLS
    OFF_WO = OFF_WQ + WQ_COLS
    OFF_AUXB = OFF_WO + NH * H
    INB_COLS = OFF_AUXB + 640
    inb = nc.dram_tensor("inb", [P, INB_COLS], BF16, kind="ExternalInput").ap()
    # auxf cols: bqT[0:16] bkT[16:20] cosq[20:532] sinq[532:1044]
    auxf = nc.dram_tensor("auxf", [D, 1044], F32, kind="ExternalInput").ap()
    hsQ = inb[:, :HSQ_COLS]
    wk = inb[:, OFF_WK : OFF_WK + WK_COLS]
    wv = inb[:, OFF_WV : OFF_WV + WK_COLS]
    wq = inb[:, OFF_WQ : OFF_WQ + WQ_COLS]
    wo = inb[:, OFF_WO : OFF_WO + NH * H]
    auxb = inb[:, OFF_AUXB : OFF_AUXB + 640]
    bqT = auxf[:, 0:NH]
    bkT = auxf[:, NH : NH + NKV]
    cosq = auxf[:, 20:532]
    sinq = auxf[:, 532:1044]
    rmat = auxb[:, 0:D]
    bv = auxb[0:1, D : D + NKV * D]
    out = nc.dram_tensor("out", [SQ, H], F32, kind="ExternalOutput").ap()

    with tile.TileContext(nc) as tc:
        # ---- long-lived left-side pools ----
        cst = tc.alloc_tile_pool(name="cst", bufs=1)
        kvq = tc.alloc_tile_pool(name="kvq", bufs=1)
        ctxp = tc.alloc_tile_pool(name="ctxp", bufs=1)

        ones1 = cst.tile([1, P], BF16, tag="ones1")
        nc.gpsimd.memset(ones1[:], 1.0)
        ones128 = cst.tile([P, P], BF16, tag="ones128")
        nc.gpsimd.memset(ones128[:], 1.0)
        bqT_sb = cst.tile([D, NH], F32, tag="bqT")
        nc.scalar.dma_start(bqT_sb[:], bqT)
        bkT_sb = cst.tile([D, NKV], F32, tag="bkT")
        nc.scalar.dma_start(bkT_sb[:], bkT)
        bv_sb = cst.tile([1, NKV * D], BF16, tag="bv")
        nc.scalar.dma_start(bv_sb[:], bv)
        r_sb = cst.tile([D, D], BF16, tag="rmat")
        nc.scalar.dma_start(r_sb[:], rmat)
        cosq_sb = cst.tile([D, SQ], F32, tag="cosq")
        nc.scalar.dma_start(cosq_sb[:], cosq)
        sinq_sb = cst.tile([D, SQ], F32, tag="sinq")
        nc.scalar.dma_start(sinq_sb[:], sinq)

        # persistent bf16 intermediates
        qt = [kvq.tile([D, SQ], BF16, tag=f"qt{h}", name=f"qt{h}") for h in range(NH)]
        kt = [kvq.tile([D, S], BF16, tag=f"kt{g}", name=f"kt{g}") for g in range(NKV)]
        vt = [kvq.tile([P, NKV * D], BF16, tag=f"v{t}", name=f"v{t}") for t in range(ST)]
        ctx = [ctxp.tile([D, SQ], BF16, tag=f"ctx{h}", name=f"ctx{h}") for h in range(NH)]

        # ---- phase 1 ----
        # Each core computes K/V only for its own 512-row seq quarter
        # (which is exactly its hsQ slice), ropes it with the same tables
        # as Q, then an inter-core DRAM AllGather over the 4-core batch
        # group assembles the full K/V while the Q projection computes.
        # This removes the 4x-redundant K/V work AND the whole hsT load.
        hsp = tc.alloc_tile_pool(name="hsp", bufs=1, side="right")
        ws1 = tc.alloc_tile_pool(name="ws1", bufs=1, side="right")
        hsq = hsp.tile([P, NT * SQ], BF16, tag="hsq", name="hsq")

        # One PSUM pool set for ALL phases (3+3+2 banks): re-allocating
        # pools at a phase boundary coalesces the WAR dependency into an
        # engine-counter barrier over the whole previous phase.
        pa = tc.alloc_tile_pool(name="ps_a", bufs=3, space="PSUM")
        pc = tc.alloc_tile_pool(name="ps_c", bufs=3, space="PSUM")
        pob = tc.alloc_tile_pool(name="ps_ob", bufs=2, space="PSUM")
        rsc = tc.alloc_tile_pool(name="rsc", bufs=2, side="right")
        dcc = tc.alloc_tile_pool(name="dcc", bufs=1, space="DRAM")
        kv_in = dcc.tile([P, 8 * 512], BF16, tag="kvin", name="kv_in")
        kv_out = dcc.tile([4 * P, 8 * 512], BF16, tag="kvout", name="kv_out")

        def bank6(i, name):
            """Alternate pa/pc so up to 6 accumulators are live at once."""
            pool = pa if i % 2 == 0 else pc
            return pool.tile([P, 512], F32, tag="pj", name=name)

        def rope(dst, tbl_cos, tbl_sin):
            """dst [128, 512] bf16, in place; rh = R^T @ x on the PE."""
            rh = pob.tile([P, 512], F32, tag="po", name="rh")
            nc.tensor.matmul(rh[:], r_sb[:], dst[:], start=True, stop=True)
            t1 = rsc.tile([P, 512], F32, tag="rt1", bufs=2, name="rt1")
            nc.vector.tensor_mul(t1[:], rh[:], tbl_sin)
            t2 = rsc.tile([P, 512], F32, tag="rt2", bufs=2, name="rt2")
            nc.vector.tensor_mul(t2[:], dst[:], tbl_cos)
            nc.vector.tensor_add(dst[:], t1[:], t2[:])

        # -- K local quarter: kt_loc[g] [d, 512] = (hsq @ Wk + bk)^T --
        klo = [
            ws1.tile([D, 512], BF16, tag=f"klo{g}", name=f"klo{g}")
            for g in range(NKV)
        ]
        vlo = [
            ws1.tile([P, NKV * D], BF16, tag=f"vlo{t}", name=f"vlo{t}")
            for t in range(4)
        ]
        # wk/wv resident (4KB/partition each): K-local's per-step DMA is
        # then just the hsq chunk (0.73us) vs 0.87us of PE work - PE-bound.
        # Chunks interleave with the hsq stream so nothing waits long.
        wkres = ws1.tile([P, NT * NKV * D], BF16, tag="wkres", name="wkres")
        wvres = ws1.tile([P, NT * NKV * D], BF16, tag="wvres", name="wvres")
        kbanks = {g: bank6(g, f"pk{g}") for g in range(NKV)}
        for ht in range(NT):
            if ht % 4 == 0:
                ck = 4 * NKV * D
                i = ht // 4
                nc.sync.dma_start(
                    wkres[:, i * ck : (i + 1) * ck],
                    wk[:, i * ck : (i + 1) * ck],
                )
            nc.sync.dma_start(
                hsq[:, ht * SQ : (ht + 1) * SQ],
                hsQ[:, ht * SQ : (ht + 1) * SQ],
            )
            for g in range(NKV):
                nc.tensor.matmul(
                    kbanks[g][:],
                    wkres[:, ht * NKV * D + g * D : ht * NKV * D + (g + 1) * D],
                    hsq[:, ht * SQ : (ht + 1) * SQ],
                    start=(ht == 0),
                    stop=(ht == NT - 1),
                )
        for g in range(NKV):
            nc.scalar.activation(
                klo[g][:], kbanks[g][:], AF.Identity,
                bias=bkT_sb[:, g : g + 1],
            )
            rope(klo[g][:], cosq_sb[:], sinq_sb[:])

        # -- V local quarter: vt_loc[ti] [128, 4*128] = hsq_sub @ Wv + bv --
        for i in range(4):
            ck = 4 * NKV * D
            nc.sync.dma_start(
                wvres[:, i * ck : (i + 1) * ck], wv[:, i * ck : (i + 1) * ck]
            )
        vbanks = {ti: bank6(ti, f"pv{ti}") for ti in range(4)}
        for ht in range(NT):
            for ti in range(4):
                nc.tensor.matmul(
                    vbanks[ti][:],
                    hsq[:, ht * SQ + ti * P : ht * SQ + (ti + 1) * P],
                    wvres[:, ht * NKV * D : (ht + 1) * NKV * D],
                    start=(ht == 0),
                    stop=False,
                )
        for ti in range(4):
            nc.tensor.matmul(
                vbanks[ti][:], ones1[:], bv_sb[:], start=False, stop=True
            )
            nc.scalar.copy(vlo[ti][:], vbanks[ti][:])

        # wq resident (64KB/partition fits now that hsT is gone): chunked
        # behind the K/V streams so Q's first matmul still starts on time,
        # and sweeps 2/3 run with zero DMA waits
        wqres = ws1.tile([P, NT * NH * D], BF16, tag="wqres", name="wqres")
        for i in range(8):
            cs = NT * NH * D // 8
            nc.sync.dma_start(
                wqres[:, i * cs : (i + 1) * cs], wq[:, i * cs : (i + 1) * cs]
            )

        # -- gather: pack -> AllGather over the batch group -> unpack.
        # All on the (otherwise idle) Pool/gpsimd queue; the Q projection
        # below overlaps the whole exchange.
        for g in range(NKV):
            nc.gpsimd.dma_start(kv_in[:, g * 512 : (g + 1) * 512], klo[g][:])
        for ti in range(4):
            nc.gpsimd.dma_start(
                kv_in[:, (4 + ti) * 512 : (5 + ti) * 512], vlo[ti][:]
            )
        if mock_cc:
            # sim-only stand-in with the same DMA volume (TimelineSim
            # cannot run collectives): broadcast own quarter to all blocks
            for k in range(4):
                nc.gpsimd.dma_start(kv_out[k * P : (k + 1) * P, :], kv_in[:])
        else:
            nc.gpsimd.collective_compute(
                "AllGather",
                mybir.AluOpType.bypass,
                replica_groups=[[0, 1, 2, 3], [4, 5, 6, 7]],
                ins=[kv_in[:].opt()],
                outs=[kv_out[:].opt()],
            )
        # unpack in attention-consumption order: kt[g] then the vt tiles
        # the first heads of that group touch, so h=0 starts ASAP
        for g in range(NKV):
            for k in range(4):
                nc.gpsimd.dma_start(
                    kt[g][:, k * 512 : (k + 1) * 512],
                    kv_out[k * P : (k + 1) * P, g * 512 : (g + 1) * 512],
                )
            for ti in range(4):
                k, tsub = divmod(g * 4 + ti, 4)
                nc.gpsimd.dma_start(
                    vt[g * 4 + ti][:],
                    kv_out[
                        k * P : (k + 1) * P, (4 + tsub) * 512 : (5 + tsub) * 512
                    ],
                )

        # -- Q: three sweeps of <=6 heads; wq streamed in per-sweep chunks --
        for h0, h1 in ((0, 6), (6, 12), (12, 16)):
            banks = {
                h: bank6(h - h0, f"pq{h}") for h in range(h0, h1)
            }
            for ht in range(NT):
                for h in range(h0, h1):
                    nc.tensor.matmul(
                        banks[h][:],
                        wqres[:, ht * NH * D + h * D : ht * NH * D + (h + 1) * D],
                        hsq[:, ht * SQ : (ht + 1) * SQ],
                        start=(ht == 0),
                        stop=(ht == NT - 1),
                    )
            for h in range(h0, h1):
                nc.scalar.activation(
                    qt[h][:], banks[h][:], AF.Identity,
                    bias=bqT_sb[:, h : h + 1],
                )
                rope(qt[h][:], cosq_sb[:], sinq_sb[:])
        rsc.release()
        ws1.release()
        hsp.release()

        # ---- phase 2: attention; wo prefetch streams during it ----
        wop = tc.alloc_tile_pool(name="wop", bufs=1, side="right")
        wores = wop.tile([P, NH * H], BF16, tag="wo", name="wores")
        for i in range(4):
            cs = NH * H // 4
            nc.sync.dma_start(
                wores[:, i * cs : (i + 1) * cs], wo[:, i * cs : (i + 1) * cs]
            )

        # out_acc[s1t] [128, 2048] f32 accumulates the output projection in
        # SBUF; groups of 4 heads go PSUM -> (DVE add) -> out_acc, so the
        # out-proj matmuls ride the PE's slack in the Act-bound attention
        # phase and there is no separate phase 3.
        oacc = tc.alloc_tile_pool(name="oacc", bufs=1, side="right")
        out_acc = [
            oacc.tile([P, H], F32, tag=f"oa{s}", name=f"oa{s}")
            for s in range(4)
        ]

        wsa = tc.alloc_tile_pool(name="wsa", bufs=4, side="right")
        DG = 8  # den group: sum DG exp-tiles on the DVE, 1 den matmul/group

        # Deferred out-proj emitters: each is one PE matmul (or one DVE
        # drain); they are popped one per (h, t) step so they fill the PE's
        # slack without ever blocking the Act-engine exp stream.
        pending = []

        def push_group_out(h):
            """Queue out-proj work for finished heads hlo..h."""
            hlo = h - 3
            for s1t in range(4):
                for hc in range(4):
                    bank_box = []

                    def mk(hh, hc=hc, s1t=s1t, h=h, hlo=hlo, bank_box=bank_box):
                        def emit():
                            if not bank_box:
                                bank_box.append(
                                    pob.tile(
                                        [P, 512], F32, tag="po", name="po"
                                    )
                                )
                            nc.tensor.matmul(
                                bank_box[0][:],
                                ctx[hh][:, s1t * P : (s1t + 1) * P],
                                wores[
                                    :,
                                    hh * H + hc * 512 : hh * H + (hc + 1) * 512,
                                ],
                                start=(hh == hlo),
                                stop=(hh == h),
                            )

                        return emit

                    for hh in range(hlo, h + 1):
                        emit = mk(hh)
                        emit.is_mm = True
                        pending.append(emit)

                    def drain(hc=hc, s1t=s1t, h=h, bank_box=bank_box):
                        dst = out_acc[s1t][:, hc * 512 : (hc + 1) * 512]
                        if h == 3:
                            nc.vector.tensor_copy(dst, bank_box[0][:])
                        else:
                            nc.vector.tensor_add(dst, dst, bank_box[0][:])

                    pending.append(drain)
                    if h == NH - 1:
                        def outdma(s1t=s1t, hc=hc):
                            nc.sync.dma_start(
                                out[
                                    s1t * P : (s1t + 1) * P,
                                    hc * 512 : (hc + 1) * 512,
                                ],
                                out_acc[s1t][:, hc * 512 : (hc + 1) * 512],
                            )

                        pending.append(outdma)

        def pop_pending():
            """Emit queued DVE drains freely plus one PE matmul."""
            while pending:
                fn = pending.pop(0)
                fn()
                if getattr(fn, "is_mm", False):
                    break

        for h in range(NH):
            g = h // GROUPS
            ctx_ps = pc.tile([P, SQ], F32, tag="pj", name=f"ctxps{h}")
            den_ps = pc.tile([P, SQ], F32, tag="pj", name=f"denps{h}")
            sc = [None] * ST

            def score(t):
                sc[t] = pa.tile([P, SQ], F32, tag="pj", name="sc")
                nc.tensor.matmul(
                    sc[t][:],
                    kt[g][:, t * P : (t + 1) * P],
                    qt[h][:],
                    start=True,
                    stop=True,
                )

            score(0)
            score(1)
            asum = None
            for t in range(ST):
                at = wsa.tile([P, SQ], BF16, tag="at", name="at")
                nc.scalar.activation(at[:], sc[t][:], AF.Exp, scale=SCALE)
                if t + 2 < ST:
                    score(t + 2)
                nc.tensor.matmul(
                    ctx_ps[:],
                    vt[t][:, g * D : (g + 1) * D],
                    at[:],
                    start=(t == 0),
                    stop=(t == ST - 1),
                )
                if t % DG == 0:
                    at0 = at
                elif t % DG == 1:
                    asum = wsa.tile([P, SQ], BF16, tag="as", bufs=2, name="asum")
                    nc.vector.tensor_add(asum[:], at0[:], at[:])
                else:
                    nc.vector.tensor_add(asum[:], asum[:], at[:])
                if t % DG == DG - 1:
                    nc.tensor.matmul(
                        den_ps[:],
                        ones128[:],
                        asum[:],
                        start=(t == DG - 1),
                        stop=(t == ST - 1),
                    )
                pop_pending()
            rc = wsa.tile([P, SQ], F32, tag="rc", bufs=2, name="rc")
            nc.vector.reciprocal(rc[:], den_ps[:])
            nc.vector.tensor_mul(ctx[h][:], ctx_ps[:], rc[:])

            if h % 4 == 3:
                push_group_out(h)
        while pending:
            pending.pop(0)()
        wsa.release()
        oacc.release()
        wop.release()
        dcc.release()
        pob.release()
        pc.release()
        pa.release()
        ctxp.release()
        kvq.release()
        cst.release()

    nc.compile()
    return nc


_PROGRAM_CACHE = {}


def _get_program():
    if "nc" not in _PROGRAM_CACHE:
        _PROGRAM_CACHE["nc"] = _build_program()
    return _PROGRAM_CACHE["nc"]


def _tile_rows(x_b):
    """[n*128, cols] -> [128, n*cols] with x_t[p, i*cols + c] = x[i*128+p, c]."""
    n = x_b.shape[0] // P
    cols = x_b.shape[1]
    return np.ascontiguousarray(
        x_b.reshape(n, P, cols).transpose(1, 0, 2).reshape(P, n * cols)
    )


def _prepare_in_maps(hidden_states, Wq, bq, Wk, bk, Wv, bv, Wo):
    bf16 = _np_bf16()
    hidden_states = np.asarray(hidden_states, dtype=np.float32)
    Wq_t = _tile_rows(np.asarray(Wq, dtype=np.float32).astype(bf16))
    Wk_t = _tile_rows(np.asarray(Wk, dtype=np.float32).astype(bf16))
    Wv_t = _tile_rows(np.asarray(Wv, dtype=np.float32).astype(bf16))
    Wo_t = _tile_rows(np.asarray(Wo, dtype=np.float32).astype(bf16))
    
    bq = np.asarray(bq, dtype=np.float32)
    bk = np.asarray(bk, dtype=np.float32)
    bv_b = np.asarray(bv, dtype=np.float32).astype(bf16).reshape(1, NKV * D)

    cosT, sinT = _rope_tables_T()
    rmat = _rotate_half_matrix().astype(bf16)
    bqT_h = np.ascontiguousarray(bq.reshape(NH, D).T)    # [128, 16]
    bkT_h = np.ascontiguousarray(bk.reshape(NKV, D).T)   # [128, 4]
    auxb_h = np.zeros((P, 640), bf16)
    auxb_h[:, :D] = rmat
    auxb_h[0:1, D : D + NKV * D] = bv_b

    hsT_b = [
        np.ascontiguousarray(hidden_states[b].T).astype(bf16) for b in range(B)
    ]

    in_maps = []
    for core in range(NCORES):
        b, tq = core // 4, core % 4
        qoff = tq * SQ
        auxf_h = np.zeros((D, 1044), np.float32)
        auxf_h[:, :NH] = bqT_h
        auxf_h[:, NH : NH + NKV] = bkT_h
        auxf_h[:, 20:532] = cosT[:, qoff : qoff + SQ]
        auxf_h[:, 532:1044] = sinT[:, qoff : qoff + SQ]
        hsq_t = _tile_rows(np.ascontiguousarray(hsT_b[b][:, qoff : qoff + SQ]))
        in_maps.append(
            {
                "inb": np.concatenate(
                    [hsq_t, Wk_t, Wv_t, Wq_t, Wo_t, auxb_h], axis=1
                ),
                "auxf": auxf_h,
            }
        )
    return in_maps


def kernel(hidden_states, Wq, bq, Wk, bk, Wv, bv, Wo):
    from concourse.bass_utils import run_bass_kernel_spmd

    in_maps = _prepare_in_maps(hidden_states, Wq, bq, Wk, bk, Wv, bv, Wo)
    nc = _get_program()
    res = run_bass_kernel_spmd(
        nc, in_maps, core_ids=list(range(NCORES)), trace=False
    )

    out_full = np.empty((B, S, H), dtype=np.float32)
    for core in range(NCORES):
        b, tq = core // 4, core % 4
        out_full[b, tq * SQ : (tq + 1) * SQ, :] = res.results[core]["out"]
    return out_full
